# revision 18
# baseline (speedup 1.0000x reference)
"""CoPEGate Trainium2 kernel.

Computes out[b,h,t,s] = sigmoid((Q K^T)[b,h,t,s] / sqrt(D)) * (P P^T)[t,s] / sqrt(D)
for B=2, H=12, T=2048, D=64 (fp32 in/out), distributed over 8 NeuronCores.

Sharding: the 24 (b,h) pairs are split 3-per-core (head-parallel); the
positional matrix P is replicated and its T x T bias is computed on every
core (reused across that core's 3 heads). No cross-device communication.

Design (all constants HW-measured on this part):

1. fp16 output. The harness tolerance is rel-err 2e-2 (L2); writing the
   output as fp16 (adds ~3e-4 L2 rounding, upcast on host) halves output
   DMA from 48 to 24 MiB/core and moves the bound from HBM writes
   (~147 us floor) to the ACT engine's sigmoid throughput.

2. K=128 matmuls. A K=64 [64x128]@[64x512] fp16 chunk matmul streams at
   427 ns (the PE clock governor holds 1.2 GHz for half-array work);
   K=128 runs at ~235-258 ns (2.4 GHz). All stationary operands are
   zero-padded to 128 contraction rows on the host (zero rows contribute
   exactly 0): lhsT sets [q0;0], [0;q1], [0;q2], [p;0]; the moving tiles
   pack two real operands each ([k0;k1], [p;k2]) so no moving bandwidth
   is wasted. PE per-core drops from ~109 us to ~60 us.

3. Half-width PSUM stripes [128,1024] (2 banks x 4 buffers = all 8
   banks): a half-stripe's matmul->sigmoid round trip (~1.6 us) never
   gates ACT, which measures wall-to-wall 1087 ns/half-sigmoid (full
   2048-wide stripes in a 2-buffer ring measure ~1.1 us/tile of refill
   bubble, a net loss). Sigmoids are the pacer: 96 x 1087 ~= 104 us.

4. DVE relief: muls run FULL-width (1226 ns vs 2x692 for halves), pos
   casts half-width (subtile-frees PSUM banks early), all on DVE:
   48x1226 + 32x1223 ~= 98 us < ACT. GPSIMD stays idle: its tensor ops
   share SBUF ports with DVE (a concurrent GPSIMD multiply measured a
   7x slowdown of DVE tensor ops).

5. Ramp: inputs arrive as SIX 512 KiB DMAs (QZ[4] lhsT sets + RHS[2]
   moving sets, zeros baked on host) ordered by first use -- v4's 11
   small DMAs serialized ~650 ns each on the SP engine and semaphore
   recycling stretched the ramp to 23 us.

Steady-state per row-tile (16 tiles), engine program order:
  PE : s1a s1b s0a s0b pp_a' s2a s2b pp_b'     (2x 512-col chunks each)
  ACT: sig1a sig1b sig0a sig0b sig2a sig2b     (1087 ns each; pacer)
  DVE: mul1 cast_a' mul0 cast_b' mul2          (pos for tile it+1)
  DMA: 3x 512 KiB output stripes
Precision: q/k/p fp16 (pos pre-scaled by D**-0.25 on host), fp16 out;
rel err ~5e-4 vs the 2e-2 gate.
"""

import math
import os
import sys

import numpy as np

sys.path.insert(0, "/opt/trn_rl_repo")

B, H, T, D = 2, 12, 2048, 64
N_CORES = 8
HPC = (B * H) // N_CORES  # heads per core
PT = 128  # output row-tile height (SBUF/PSUM partitions)
NT = T // PT  # row tiles
NCHUNK = 512  # matmul moving-operand free dim (one PSUM bank of fp32)
NCH = T // NCHUNK
HW = T // 2  # half-stripe width: [128, HW] f32 = 2 PSUM banks
INV_SQRT_D = 1.0 / math.sqrt(D)

_NC_CACHE = {}


def _build_nc():
    import concourse.bass as bass
    from concourse import bacc, mybir, tile

    f32 = mybir.dt.float32
    f16 = mybir.dt.float16
    Sigmoid = mybir.ActivationFunctionType.Sigmoid

    nc = bacc.Bacc("TRN2", target_bir_lowering=False)

    # Host-packed operands (see module docstring):
    #   QZ[s] = the REAL 64 rows of stationary set s (q0, q1, q2, p);
    #   the other 64 rows of each [128, T] SBUF tile are DVE-memset to 0
    #   (sending baked zeros would add 1 MiB to the ramp-critical wire).
    #   RHS[0]=[k0;k1] RHS[1]=[p;k2]  (moving, both halves real)
    QZ = nc.dram_tensor("QZ", [4, D, T], f16, kind="ExternalInput")
    RHS = nc.dram_tensor("RHS", [2, 2 * D, T], f16, kind="ExternalInput")
    out = nc.dram_tensor("out", [HPC, T, T], f16, kind="ExternalOutput")

    with tile.TileContext(nc) as tc:
        with tc.tile_pool(name="ins", bufs=1) as ins_pool, \
             tc.tile_pool(name="pos", bufs=3) as pos_pool, \
             tc.tile_pool(name="gate", bufs=6) as gate_pool, \
             tc.tile_pool(name="outs", bufs=12) as outs_pool, \
             tc.tile_pool(name="ps", bufs=4, space="PSUM") as ps_pool:

            # One [128, T] SBUF tile per operand set. The aggregate input
            # wire (~2 MiB at ~360 GB/s) bounds the ramp, so DMAs carry
            # only the real halves (zero halves are DVE-memset -- DVE is
            # idle until ~12 us) and issue in first-use order on SP.
            qz1 = ins_pool.tile([2 * D, T], f16, tag="qz1")
            rk = ins_pool.tile([2 * D, T], f16, tag="rk")
            qz3 = ins_pool.tile([2 * D, T], f16, tag="qz3")
            rp = ins_pool.tile([2 * D, T], f16, tag="rp")
            qz0 = ins_pool.tile([2 * D, T], f16, tag="qz0")
            qz2 = ins_pool.tile([2 * D, T], f16, tag="qz2")
            nc.vector.memset(qz1[0:D, :], 0.0)
            nc.vector.memset(qz3[D : 2 * D, :], 0.0)
            nc.gpsimd.memset(qz0[D : 2 * D, :], 0.0)
            nc.gpsimd.memset(qz2[0:D, :], 0.0)
            nc.sync.dma_start(out=qz1[D : 2 * D, :], in_=QZ[1][:, :])
            nc.sync.dma_start(out=rk[:, 0:HW], in_=RHS[0][:, 0:HW])
            nc.sync.dma_start(out=rk[:, HW:], in_=RHS[0][:, HW:])
            nc.sync.dma_start(out=qz0[0:D, :], in_=QZ[0][:, :])
            nc.sync.dma_start(out=qz3[0:D, :], in_=QZ[3][:, :])
            nc.sync.dma_start(out=rp, in_=RHS[1][:, :])
            nc.sync.dma_start(out=qz2[D : 2 * D, :], in_=QZ[2][:, :])

            lhs_t = {0: qz0, 1: qz1, 2: qz2, 3: qz3}
            rhs_t = {0: rk, 1: rk, 2: rp, 3: rp}

            def mm_half(psum, s, it, half):
                # Fill one [128, HW] half-stripe = 2 one-bank matmuls.
                lhsT = lhs_t[s][:, bass.ts(it, PT)]
                for jj in range(2):
                    j = 2 * half + jj
                    nc.tensor.matmul(
                        psum[:, bass.ts(jj, NCHUNK)],
                        lhsT,
                        rhs_t[s][:, bass.ts(j, NCHUNK)],
                        start=True,
                        stop=True,
                    )

            def pos_half(pos_sb, it, half):
                # pos half-stripe for tile `it`: matmul + DVE cast f32->f16.
                pp = ps_pool.tile([PT, HW], f32, tag="ps")
                mm_half(pp, 3, it, half)
                nc.vector.tensor_copy(pos_sb[:, bass.ts(half, HW)], pp)

            def head_sig(h, it):
                # Score half-stripes + half-width sigmoids -> full gate.
                gate = gate_pool.tile([PT, T], f16, tag="gate")
                for half in range(2):
                    sp = ps_pool.tile([PT, HW], f32, tag="ps")
                    mm_half(sp, h, it, half)
                    nc.scalar.activation(
                        gate[:, bass.ts(half, HW)], sp, Sigmoid, scale=INV_SQRT_D
                    )
                return gate

            def head_mul_dma(h, it, gate, pos_sb, split=False):
                o = outs_pool.tile([PT, T], f16, tag="o")
                if split:
                    # Tail trim: half-width muls + DMAs so the final
                    # bytes trail the last sigmoid by ~1.5 us, not ~2.7.
                    for half in range(2):
                        hsl = bass.ts(half, HW)
                        nc.vector.tensor_mul(o[:, hsl], gate[:, hsl], pos_sb[:, hsl])
                        nc.sync.dma_start(
                            out=out[h, bass.ts(it, PT), hsl], in_=o[:, hsl]
                        )
                else:
                    nc.vector.tensor_mul(o, gate, pos_sb)
                    nc.sync.dma_start(out=out[h, bass.ts(it, PT), :], in_=o)

            # ---- tile 0: score stripes for heads 1+0 BEFORE the pos
            # prologue (their inputs land first and ACT is the pacer,
            # while pos is only needed by the muls, which have slack).
            sp1, sp0 = [], []
            for half in range(2):
                sp = ps_pool.tile([PT, HW], f32, tag="ps")
                mm_half(sp, 1, 0, half)
                sp1.append(sp)
            for half in range(2):
                sp = ps_pool.tile([PT, HW], f32, tag="ps")
                mm_half(sp, 0, 0, half)
                sp0.append(sp)
            pos_cur = pos_pool.tile([PT, T], f16, tag="pos")
            for half in range(2):
                pos_half(pos_cur, 0, half)

            def sig_prefilled(sps):
                gate = gate_pool.tile([PT, T], f16, tag="gate")
                for half in range(2):
                    nc.scalar.activation(
                        gate[:, bass.ts(half, HW)], sps[half],
                        Sigmoid, scale=INV_SQRT_D,
                    )
                return gate

            # ---- tiles ------------------------------------------------
            # PSUM ring (4 bufs): s1a s1b s0a s0b pp_a' s2a s2b pp_b'
            # -> every sigmoid's refill has >= 850 ns slack; pos for tile
            # it+1 is produced between this tile's muls on DVE.
            for it in range(NT):
                gate1 = sig_prefilled(sp1) if it == 0 else head_sig(1, it)
                if it == 0:
                    gate0 = sig_prefilled(sp0)
                    pos_next = pos_pool.tile([PT, T], f16, tag="pos")
                    pos_half(pos_next, 1, 0)
                    head_mul_dma(1, it, gate1, pos_cur)
                else:
                    head_mul_dma(1, it, gate1, pos_cur)
                    gate0 = head_sig(0, it)
                    pos_next = None
                    if it + 1 < NT:
                        pos_next = pos_pool.tile([PT, T], f16, tag="pos")
                        pos_half(pos_next, it + 1, 0)
                head_mul_dma(0, it, gate0, pos_cur)

                gate2 = head_sig(2, it)
                if pos_next is not None:
                    pos_half(pos_next, it + 1, 1)
                head_mul_dma(2, it, gate2, pos_cur, split=(it == NT - 1))
                if pos_next is not None:
                    pos_cur = pos_next

    nc.finalize()
    return nc


def _get_nc():
    if "nc" not in _NC_CACHE:
        _NC_CACHE["nc"] = _build_nc()
    return _NC_CACHE["nc"]


def kernel(query, key, pos_embed_weight):
    query = np.asarray(query, dtype=np.float32)
    key = np.asarray(key, dtype=np.float32)
    pos_embed_weight = np.asarray(pos_embed_weight, dtype=np.float32)

    q = query.reshape(B * H, T, D)
    k = key.reshape(B * H, T, D)
    # Fold the pos-bias 1/sqrt(D) into the (replicated) P operand: the
    # matmul computes (s*P)(s*P)^T = P P^T / sqrt(D) with s = D**-0.25.
    p_t = (pos_embed_weight[:T].T * np.float32(D**-0.25)).astype(np.float16)

    in_maps = []
    for c in range(N_CORES):
        h0, h1, h2 = c * HPC, c * HPC + 1, c * HPC + 2
        qT = [
            np.ascontiguousarray(q[h].T).astype(np.float16)
            for h in (h0, h1, h2)
        ]
        kT = [
            np.ascontiguousarray(k[h].T).astype(np.float16)
            for h in (h0, h1, h2)
        ]
        qz = np.empty((4, D, T), dtype=np.float16)
        qz[0] = qT[0]
        qz[1] = qT[1]
        qz[2] = qT[2]
        qz[3] = p_t
        rhs = np.empty((2, 2 * D, T), dtype=np.float16)
        rhs[0, :D] = kT[0]
        rhs[0, D:] = kT[1]
        rhs[1, :D] = p_t
        rhs[1, D:] = kT[2]
        in_maps.append({"QZ": qz, "RHS": rhs})

    from concourse.bass_utils import run_bass_kernel_spmd

    nc = _get_nc()
    try:
        res = run_bass_kernel_spmd(
            nc,
            in_maps,
            core_ids=list(range(N_CORES)),
            trace=bool(os.environ.get("KERNEL_TRACE")),
        )
    except Exception:
        # One retry for transient runtime/compile hiccups.
        res = run_bass_kernel_spmd(
            nc, in_maps, core_ids=list(range(N_CORES)), trace=False
        )
    kernel.last_results = res

    full = np.empty((B * H, T, T), dtype=np.float32)
    for c in range(N_CORES):
        full[c * HPC : (c + 1) * HPC] = res.results[c]["out"]
    return full.reshape(B, H, T, T)


kernel.last_results = None
